# revision 53
# baseline (speedup 1.0000x reference)
"""AttnBlock (GroupNorm -> q/k/v 1x1 conv -> full spatial attention -> out proj
-> residual) for Trainium2, sharded over 8 NeuronCores.

Sharding: 8 cores = 4 batches x 2 query-halves. Each core gets its batch's
full x (columns rotated so its 2048 query positions come first), computes
GroupNorm + k/v over all 4096 positions (k/v redundantly per half) and
attention output for its 2048 queries.

On-chip design (per core), all big matmuls in fp8-e4m3 with DoubleRow perf
mode (contracts 256 rows/instruction = 2x the f32r rate; measured 223ns per
[256K x 128 x 512] instruction vs 233ns for f32r's 128K):
  - x is DMA'd once into SBUF and stays resident (stats pass reads SBUF-bound
    tiles as they land; projections and the residual reuse it -> no second
    HBM pass, no q DRAM scratch).
  - weights are quantized to fp8 on the host directly in the DoubleRow pair
    layout [h][c128, 2, o] and DMA'd straight to SBUF.
  - xn is quantized to fp8 by DVE tensor_scalar (gscale*x+gshift) into pair
    layout; q/k (feature-major) and vT (token-major) projections are
    DoubleRow matmuls; PSUM tiles drain to fp8 SBUF via ACT/DVE copies.
  - attention per 512-query group: logits (2 insts/jc) -> ACT exp with
    bias -3 (softmax-invariant shift keeping exp in e4m3 range) written as
    fp8 -> PV DoubleRow over jc pairs into 4 held PSUM banks. The softmax
    denominator accumulates on the PE via a 32-column ones DoubleRow
    matmul per pair (frees DVE from the inner loop; M=1 stationaries are
    illegal in dual-fp8). Logit fronts are emitted two pairs ahead of
    their dps/pv consumers so the PE never waits on ACT (99-100% PE busy
    in steady state). pv drains are scaled by 2^-6 into fp8; normalization
    (64/denom, folded into the ones=64 broadcast matmul) is deferred past
    the fp8 out-projection, then the residual is added from resident x.
    Each group's tail is spliced into the next group's matmul stream.
  - outputs DMA on gpsimd's hardware queues while attention runs (sync
    software-queue bursts steal SBUF ports from the PE and slow the matmul
    stream ~10%); the last two groups flush on sync, which drains faster.

Measured on HW: 243.7us vs the 457.7us f32r baseline (1.89x), rel err
8.2e-3 against the f32 reference (gate 2e-2).
"""

import numpy as np
import ml_dtypes

import bass_rust
import concourse.bass as bass
import concourse.tile as tile
from concourse import mybir
from concourse.bass_utils import run_bass_kernel_spmd

B, C, H, W = 4, 512, 64, 64
HW = H * W            # 4096
HALF = HW // 2        # 2048 query positions per core
NG = 32               # groups
GS = C // NG          # 16 channels per group
EPS = 1e-6
P = 128               # SBUF partitions
NCC = C // P          # 4 channel chunks
NH = NCC // 2         # 2 channel-chunk pairs (DoubleRow)
JT = 512              # projection j-tile width
NJT = HW // JT        # 8
NJC = HW // P         # 32 j-chunks of 128
NPAIR = NJC // 2      # 16 j-chunk pairs
IGW = 512             # query-group width
NIG = HALF // IGW     # 4
SCALE = 1.0 / float(np.sqrt(C))
EXP_BIAS = -3.0       # softmax-invariant logit shift: exp stays in e4m3 range
PV_SCALE = 1.0 / 64.0  # raw-pv prescale into fp8; 64 folded into ones-bcast
F32 = mybir.dt.float32
F32R = mybir.dt.float32r
F8 = mybir.dt.float8e4
DR = mybir.MatmulPerfMode.DoubleRow
E4NP = ml_dtypes.float8_e4m3

AF = mybir.ActivationFunctionType
ALU = mybir.AluOpType


def _split_drain_waits(nc, max_waits=1):
    """walrus on this container rejects ANY instruction carrying more than one
    sem wait; spill the excess onto same-engine NoOps inserted just before
    (the engine executes the NoOp's waits, then the instruction's remaining
    one -- identical semantics)."""
    uid = [0]
    nsplit = 0
    for f in nc.m.functions:
        for bb in f.blocks:
            insts = bb.instructions
            i = 0
            while i < len(insts):
                inst = insts[i]
                si = getattr(inst, "sync_info", None)
                if si is not None and si.on_wait and len(si.on_wait) > max_waits:
                    waits = list(si.on_wait)
                    keep, rest = waits[-max_waits:], waits[:-max_waits]
                    new_insts = []
                    for j in range(0, len(rest), max_waits):
                        nop = mybir.InstNoOp(
                            name=f"wait-split-{uid[0]}", ins=[], outs=[]
                        )
                        uid[0] += 1
                        nop.engine = inst.engine
                        nop.sync_info = bass_rust.SyncInfo(
                            on_wait=rest[j : j + max_waits], on_update=[]
                        )
                        new_insts.append(nop)
                    inst.sync_info = bass_rust.SyncInfo(
                        on_wait=keep, on_update=list(si.on_update)
                    )
                    for k, nop in enumerate(new_insts):
                        insts.insert(i + k, nop)
                    i += len(new_insts)
                    nsplit += 1
                i += 1
    return nsplit


def build():
    nc = bass.Bass()
    xb = nc.dram_tensor("xb", [C, HW], F32, kind="ExternalInput")
    # stationary (lhsT) weights: [p, oc, t, m] -- each (t, m) pair block is
    # contiguous per partition (dual-fp8 LDWEIGHTS ISA requirement)
    wq8 = nc.dram_tensor("wq8", [NH * P, NCC, 2, P], F8, kind="ExternalInput")
    wk8 = nc.dram_tensor("wk8", [NH * P, NCC, 2, P], F8, kind="ExternalInput")
    wo8 = nc.dram_tensor("wo8", [NH * P, NCC, 2, P], F8, kind="ExternalInput")
    # moving (rhs) weights for the v projection: [p, t, o]
    wv8 = nc.dram_tensor("wv8", [NH * P, 2, C], F8, kind="ExternalInput")
    gnw = nc.dram_tensor("gnw", [C], F32, kind="ExternalInput")
    gnb = nc.dram_tensor("gnb", [C], F32, kind="ExternalInput")
    # local (within-chunk) group membership: identical for every channel
    # chunk since groups never span chunks (8 groups x 16 channels per chunk)
    membd = nc.dram_tensor("membd", [P, 8], F32, kind="ExternalInput")
    bcd = nc.dram_tensor("bcd", [8, P], F32, kind="ExternalInput")
    outd = nc.dram_tensor("out", [C, HALF], F32, kind="ExternalOutput")

    with tile.TileContext(nc) as tc, nc.allow_low_precision(
        reason="fp8 matmul pipeline; rel gate 2e-2, measured ~8e-3"
    ):
        with tc.tile_pool(name="pers", bufs=1) as pers:
            # ---- persistent tiles ----
            x_sb = [pers.tile([P, HW], F32, tag=f"x{cc}", name=f"x{cc}")
                    for cc in range(NCC)]
            # stationary tiles: [p, blk, t, m] -- (t, m) contiguous per blk
            k8_sb = [pers.tile([P, NJC, 2, P], F8, tag=f"k8{h}", name=f"k8{h}")
                     for h in range(NH)]
            vT8_sb = pers.tile([P, NPAIR, NCC, 2, P], F8, tag="vT8")
            q8_sb = [pers.tile([P, 2, HALF], F8, tag=f"q8{h}", name=f"q8{h}")
                     for h in range(NH)]
            wq_sb = [pers.tile([P, NCC, 2, P], F8, tag=f"wq{h}", name=f"wq{h}")
                     for h in range(NH)]
            wk_sb = [pers.tile([P, NCC, 2, P], F8, tag=f"wk{h}", name=f"wk{h}")
                     for h in range(NH)]
            wv_sb = [pers.tile([P, 2, C], F8, tag=f"wv{h}", name=f"wv{h}")
                     for h in range(NH)]
            wo_sb = [pers.tile([P, NCC, 2, P], F8, tag=f"wo{h}", name=f"wo{h}")
                     for h in range(NH)]
            attn8 = [pers.tile([P, 2, IGW], F8, tag=f"at8{h}", name=f"at8{h}")
                     for h in range(NH)]
            xn8_first = [pers.tile([P, 8, 2, P], F8, tag=f"xnf{h}", name=f"xnf{h}")
                         for h in range(NH)]
            # weight/const DMAs: keep them OFF the scalar queue (ACT does the
            # stats Squares on the critical path); gn/memb/bc go early on
            # sync, the fp8 weights on gpsimd interleaved with x tiles.
            gnw_t = pers.tile([P, NCC], F32, tag="gnw")
            gnb_t = pers.tile([P, NCC], F32, tag="gnb")
            nc.gpsimd.dma_start(out=gnw_t, in_=gnw.ap().rearrange("(a p) -> p a", p=P))
            nc.gpsimd.dma_start(out=gnb_t, in_=gnb.ap().rearrange("(a p) -> p a", p=P))
            gscale = pers.tile([P, NCC], F32, tag="gsc")
            gshift = pers.tile([P, NCC], F32, tag="gsh")
            memb = pers.tile([P, 8], F32, tag="memb")
            nc.gpsimd.dma_start(out=memb, in_=membd.ap())
            bc = pers.tile([8, P], F32, tag="bc")
            nc.gpsimd.dma_start(out=bc, in_=bcd.ap())
            ones1r64f = pers.tile([1, P], F32, tag="ones64f")
            nc.vector.memset(ones1r64f, 64.0)
            ones1r64 = pers.tile([1, P], F32R, tag="ones64")
            nc.vector.tensor_copy(out=ones1r64, in_=ones1r64f)
            ones8f = pers.tile([P, 2, 32], F32, tag="ones8f")
            nc.vector.memset(ones8f, 1.0)
            ones8 = pers.tile([P, 2, 32], F8, tag="ones8")
            nc.vector.tensor_copy(out=ones8, in_=ones8f)
            ebias = pers.tile([P, 1], F32, tag="ebias")
            nc.vector.memset(ebias, EXP_BIAS)

            # ---- phase A: x -> SBUF once; groupnorm statistics on the fly ----
            with (
                tc.tile_pool(name="statq", bufs=2) as sq_pool,
                tc.tile_pool(name="statsm", bufs=1) as sm,
                tc.tile_pool(name="statps", bufs=1, space="PSUM") as sps,
                nc.named_scope("stats"),
            ):
                sbeps = sm.tile([8, 1], F32, tag="eps")
                nc.vector.memset(sbeps, EPS)
                for cc in range(NCC):
                    s1t = sm.tile([P, 4], F32, tag=f"s1{cc}", name=f"s1{cc}")
                    s2t = sm.tile([P, 4], F32, tag=f"s2{cc}", name=f"s2{cc}")
                    for jt in range(4):
                        # all x tiles on the sync (software-dynamic) DMA
                        # queues: they complete far sooner than the gpsimd
                        # hardware queues, and the stats chain gates on the
                        # slowest stream
                        xsl = x_sb[cc][:, jt * 1024 : (jt + 1) * 1024]
                        nc.sync.dma_start(
                            out=xsl,
                            in_=xb.ap()[
                                cc * P : (cc + 1) * P,
                                jt * 1024 : (jt + 1) * 1024,
                            ],
                        )
                        nc.vector.reduce_sum(
                            out=s1t[:, jt : jt + 1], in_=xsl, axis=mybir.AxisListType.X
                        )
                        sqw = sq_pool.tile([P, 1024], F32, tag="sqw", name="sqw")
                        nc.scalar.activation(
                            out=sqw,
                            in_=xsl,
                            func=AF.Square,
                            accum_out=s2t[:, jt : jt + 1],
                        )
                    # groups never span channel chunks (8 groups per chunk),
                    # so each chunk's gscale/gshift resolves independently --
                    # no serial all-chunk reduction at the stats tail
                    mm2 = sm.tile([P, 2], F32, tag=f"m2{cc}", name=f"m2{cc}")
                    m1r = sm.tile([P, 1], F32, tag=f"m1r{cc}", name=f"m1r{cc}")
                    nc.vector.reduce_sum(out=m1r, in_=s1t, axis=mybir.AxisListType.X)
                    nc.vector.tensor_scalar_mul(mm2[:, 0:1], m1r, 1.0 / HW)
                    m2r = sm.tile([P, 1], F32, tag=f"m2r{cc}", name=f"m2r{cc}")
                    nc.vector.reduce_sum(out=m2r, in_=s2t, axis=mybir.AxisListType.X)
                    nc.vector.tensor_scalar_mul(mm2[:, 1:2], m2r, 1.0 / HW)
                    gps = sps.tile([8, 2], F32, tag="gstat", name=f"gps{cc}")
                    nc.tensor.matmul(gps, memb, mm2, start=True, stop=True)
                    # group stats for this chunk's 8 local groups
                    gs = sm.tile([8, 2], F32, tag=f"gs{cc}", name=f"gs{cc}")
                    nc.scalar.mul(gs, gps, 1.0 / GS)
                    sqg = sm.tile([8, 1], F32, tag=f"sq{cc}", name=f"sq{cc}")
                    nc.vector.tensor_mul(sqg, gs[:, 0:1], gs[:, 0:1])
                    varg = sm.tile([8, 1], F32, tag=f"vr{cc}", name=f"vr{cc}")
                    nc.vector.tensor_sub(varg, gs[:, 1:2], sqg)
                    g2 = sm.tile([8, 2], F32, tag=f"g2{cc}", name=f"g2{cc}")
                    nc.vector.tensor_copy(g2[:, 0:1], gs[:, 0:1])
                    nc.scalar.activation(
                        out=g2[:, 1:2], in_=varg, func=AF.Sqrt, bias=sbeps
                    )
                    nc.vector.reciprocal(out=g2[:, 1:2], in_=g2[:, 1:2])
                    chp = sps.tile([P, 2], F32, tag="chs", name="chs")
                    nc.tensor.matmul(chp, bc, g2, start=True, stop=True)
                    nc.vector.tensor_mul(
                        gscale[:, cc : cc + 1], chp[:, 1:2], gnw_t[:, cc : cc + 1]
                    )
                    tmpm = sm.tile([P, 1], F32, tag="tm", name="tm")
                    nc.vector.tensor_mul(tmpm, chp[:, 0:1], gscale[:, cc : cc + 1])
                    nc.vector.tensor_sub(
                        gshift[:, cc : cc + 1], gnb_t[:, cc : cc + 1], tmpm
                    )
                    # quantize this chunk's first projection supertile now --
                    # DVE would otherwise serialize all four quants right
                    # before the first projection matmul
                    nc.vector.tensor_scalar(
                        out=xn8_first[cc // 2][:, :, cc % 2, :],
                        in0=x_sb[cc][:, 0 : 2 * JT].rearrange(
                            "p (a m) -> p a m", a=8
                        ),
                        scalar1=gscale[:, cc : cc + 1],
                        scalar2=gshift[:, cc : cc + 1],
                        op0=ALU.mult,
                        op1=ALU.add,
                    )
                # fp8 weights land behind the x stream on gpsimd (needed
                # only once projections start)
                for h in range(NH):
                    wsl = slice(h * P, (h + 1) * P)
                    nc.gpsimd.dma_start(out=wq_sb[h], in_=wq8.ap()[wsl])
                    nc.gpsimd.dma_start(out=wk_sb[h], in_=wk8.ap()[wsl])
                    nc.gpsimd.dma_start(out=wv_sb[h], in_=wv8.ap()[wsl])
                    nc.gpsimd.dma_start(out=wo_sb[h], in_=wo8.ap()[wsl])

            # ---- phase B: projections (k, vT, q), all fp8 DoubleRow ----
            with (
                tc.tile_pool(name="projxn", bufs=2) as pxn,
                tc.tile_pool(name="projps", bufs=4, space="PSUM") as pps,
                nc.named_scope("proj"),
            ):
                # drains batch an output-channel (or token-chunk) pair into
                # one [128, 1024] copy spanning two PSUM banks -- amortizes
                # the per-op access latency on ACT/DVE
                for jt2 in range(NJT // 2):
                    jsl2 = slice(jt2 * 2 * JT, (jt2 + 1) * 2 * JT)
                    # xn8[h]: [p, js(8), t, m] -- contiguous (t, m) pair
                    # blocks for the v lhsT; q/k use the permuted view.
                    # jt2==0 was already quantized during the stats tail.
                    if jt2 == 0:
                        xn8 = xn8_first
                    else:
                        xn8 = [pxn.tile([P, 8, 2, P], F8, tag=f"xn{h}", name=f"xn{h}")
                               for h in range(NH)]
                        for cc in range(NCC):
                            nc.vector.tensor_scalar(
                                out=xn8[cc // 2][:, :, cc % 2, :],
                                in0=x_sb[cc][:, jsl2].rearrange("p (a m) -> p a m", a=8),
                                scalar1=gscale[:, cc : cc + 1],
                                scalar2=gshift[:, cc : cc + 1],
                                op0=ALU.mult,
                                op1=ALU.add,
                            )
                    for half in range(2):
                        jt = jt2 * 2 + half
                        jsl = slice(jt * JT, (jt + 1) * JT)
                        xnmov = [
                            xn8[h].rearrange("p a t m -> p t a m")[
                                :, :, half * 4 : (half + 1) * 4, :
                            ]
                            for h in range(NH)
                        ]
                        # k pairs (feature-major)
                        for hp in range(NH):
                            kps = pps.tile([P, 2 * JT], F32, tag="pp", name="kps")
                            for t in range(2):
                                oc = 2 * hp + t
                                for h in range(NH):
                                    nc.tensor.matmul(
                                        kps[:, t * JT : (t + 1) * JT],
                                        wk_sb[h][:, oc, :, :],
                                        xnmov[h],
                                        start=(h == 0),
                                        stop=(h == NH - 1),
                                        perf_mode=DR,
                                    )
                            nc.scalar.copy(
                                out=k8_sb[hp][:, jt * 4 : (jt + 1) * 4, :, :],
                                in_=kps.rearrange("p (t a m) -> p a t m", t=2, a=4),
                            )
                        # v pairs (token-major)
                        for vp in range(2):
                            vg = jt * 2 + vp
                            vps = pps.tile([P, 2 * JT], F32, tag="pp", name="vps")
                            for t in range(2):
                                js8 = half * 4 + 2 * vp + t
                                for h in range(NH):
                                    nc.tensor.matmul(
                                        vps[:, t * JT : (t + 1) * JT],
                                        xn8[h][:, js8, :, :],
                                        wv_sb[h],
                                        start=(h == 0),
                                        stop=(h == NH - 1),
                                        perf_mode=DR,
                                    )
                            vdst = vT8_sb[:, vg, :, :, :]
                            vsrc = vps.rearrange("p (t a m) -> p a t m", t=2, a=4)
                            # jt2 0-1 carry the q drains on ACT, so v goes to
                            # DVE there; later supertiles split v evenly
                            if jt2 < 2 or vp == 1:
                                nc.vector.tensor_copy(out=vdst, in_=vsrc)
                            else:
                                nc.scalar.copy(out=vdst, in_=vsrc)
                        # q pairs (first half only = our queries) on DVE
                        if jt < NJT // 2:
                            for hp in range(NH):
                                qps = pps.tile([P, 2 * JT], F32, tag="pp", name="qps")
                                for t in range(2):
                                    oc = 2 * hp + t
                                    for h in range(NH):
                                        nc.tensor.matmul(
                                            qps[:, t * JT : (t + 1) * JT],
                                            wq_sb[h][:, oc, :, :],
                                            xnmov[h],
                                            start=(h == 0),
                                            stop=(h == NH - 1),
                                            perf_mode=DR,
                                        )
                                nc.scalar.copy(
                                    out=q8_sb[hp][:, :, jsl],
                                    in_=qps.rearrange("p (t m) -> p t m", t=2),
                                )

            # ---- phase C: attention + output projection + residual ----
            with (
                tc.tile_pool(name="attnex", bufs=6) as aep,
                tc.tile_pool(name="attnsm", bufs=2) as asm_,
                tc.tile_pool(name="attnfo", bufs=3) as afo,
                tc.tile_pool(name="attnap", bufs=3, space="PSUM") as ap2,
                tc.tile_pool(name="attnpv", bufs=1, space="PSUM") as pvp_pool,
                tc.tile_pool(name="attndn", bufs=1, space="PSUM") as dnp,
                nc.named_scope("attn"),
            ):
                pending = None
                for ig in range(NIG):
                    isl = slice(ig * IGW, (ig + 1) * IGW)
                    dps = dnp.tile([32, IGW], F32, tag="dps", name="dps")
                    pvp = [
                        pvp_pool.tile([P, IGW], F32, tag=f"pv{cc}", name=f"pv{cc}")
                        for cc in range(NCC)
                    ]
                    exs = {}

                    # logits + exp for one key chunk; emitted two pairs ahead
                    # of its dps/pv consumers so the PE never waits on ACT
                    def emit_front(jc, isl=isl, exs=exs):
                        if pending is not None and jc in pending:
                            pending.pop(jc)()
                        ap_t = ap2.tile([P, IGW], F32, tag="ap", name="ap_t")
                        for h in range(NH):
                            nc.tensor.matmul(
                                ap_t,
                                k8_sb[h][:, jc, :, :],
                                q8_sb[h][:, :, isl],
                                start=(h == 0),
                                stop=(h == NH - 1),
                                perf_mode=DR,
                            )
                        if jc % 2 == 0:
                            exs[jc // 2] = aep.tile(
                                [P, 2, IGW], F8, tag="ex", name="ex"
                            )
                        nc.scalar.activation(
                            out=exs[jc // 2][:, jc % 2, :],
                            in_=ap_t,
                            func=AF.Exp,
                            scale=SCALE,
                            bias=ebias,
                        )

                    # dps lags 3 pairs behind pv: with a single dps bank, the
                    # next ig's first denominator matmul then reaches the PE
                    # only after this ig's reciprocal has read the bank --
                    # no WAR stall, and the freed bank buys ap_t a 3rd buffer
                    DLAG = 3

                    def emit_dps(p, dps=dps):
                        nc.tensor.matmul(
                            dps,
                            ones8,
                            exs[p],
                            start=(p == 0),
                            stop=(p == NPAIR - 1),
                            perf_mode=DR,
                        )

                    for jc in range(4):
                        emit_front(jc)
                    for pair in range(NPAIR):
                        if pair + 2 < NPAIR:
                            emit_front(2 * pair + 4)
                            emit_front(2 * pair + 5)
                        if pair >= DLAG:
                            emit_dps(pair - DLAG)
                            exs.pop(pair - DLAG)
                        ex_pair = exs[pair]
                        for cc in range(NCC):
                            nc.tensor.matmul(
                                pvp[cc],
                                vT8_sb[:, pair, cc, :, :],
                                ex_pair,
                                start=(pair == 0),
                                stop=(pair == NPAIR - 1),
                                perf_mode=DR,
                            )
                    for p in range(NPAIR - DLAG, NPAIR):
                        emit_dps(p)
                        exs.pop(p)
                    # ig end: drain raw pv (scaled 2^-6) into fp8 on DVE;
                    # frees the 4 pv banks for the next ig's first pair.
                    # (ACT drains here stall the next ig's exp chain -- only
                    # the final ig, with no exps after it, may use ACT, which
                    # overlaps the drains with the tail's reciprocal on DVE.)
                    for cc in range(NCC):
                        adst = attn8[cc // 2][:, cc % 2, :]
                        if ig == NIG - 1 and cc % 2 == 1:
                            nc.scalar.activation(
                                out=adst, in_=pvp[cc], func=AF.Copy,
                                scale=PV_SCALE,
                            )
                        else:
                            nc.vector.tensor_scalar_mul(adst, pvp[cc], PV_SCALE)

                    def make_tail(isl=isl, dps=dps, last=(ig >= NIG - 2)):
                        recip = asm_.tile([1, IGW], F32R, tag="recip", name="recip")
                        bcs = asm_.tile([P, IGW], F32, tag="bcs", name="bcs")

                        def t_norm():
                            # 64/denom broadcast to all partitions
                            nc.vector.reciprocal(out=recip, in_=dps[0:1, :])
                            bcp = ap2.tile([P, IGW], F32, tag="ap", name="bcp")
                            nc.tensor.matmul(
                                bcp, ones1r64, recip, start=True, stop=True
                            )
                            nc.vector.tensor_copy(out=bcs, in_=bcp)

                        def t_oc(oc):
                            def f():
                                oop = ap2.tile([P, IGW], F32, tag="ap", name="oop")
                                for h in range(NH):
                                    nc.tensor.matmul(
                                        oop,
                                        wo_sb[h][:, oc, :, :],
                                        attn8[h],
                                        start=(h == 0),
                                        stop=(h == NH - 1),
                                        perf_mode=DR,
                                    )
                                tmpo = afo.tile([P, IGW], F32, tag="tmpo", name="tmpo")
                                nc.vector.tensor_mul(tmpo, oop, bcs)
                                fo = afo.tile([P, IGW], F32, tag="fout", name="fout")
                                nc.vector.tensor_add(fo, tmpo, x_sb[oc][:, isl])
                                # gpsimd hw-queues while attention still runs
                                # (sync DMA bursts steal SBUF ports from the
                                # PE); the final group flushes on sync's fast
                                # software queues instead
                                oeng = nc.sync if last else nc.gpsimd
                                oeng.dma_start(
                                    out=outd.ap()[oc * P : (oc + 1) * P, isl],
                                    in_=fo,
                                )
                            return f

                        return {
                            5: t_norm,
                            8: t_oc(0),
                            10: t_oc(1),
                            12: t_oc(2),
                            14: t_oc(3),
                        }

                    pending = make_tail()
                for jc in sorted(pending):
                    pending[jc]()

    return nc


_NC_CACHE = {}


def _get_module():
    if "nc" not in _NC_CACHE:
        nc = build()
        _split_drain_waits(nc)  # only needed for walrus codegen, not CoreSim
        _NC_CACHE["nc"] = nc
    return _NC_CACHE["nc"]


def _memb_np():
    m = np.zeros((P, 8), np.float32)
    for p in range(P):
        m[p, p // GS] = 1.0
    return m


def _bc_np():
    b = np.zeros((8, P), np.float32)
    for p in range(P):
        b[p // GS, p] = 1.0
    return b


def _pack8_stat(w):
    # stationary: [h*P+p, oc, t, m] = fp8(w.T[128*(2h+t)+p, 128*oc+m])
    wT = np.ascontiguousarray(np.asarray(w, np.float32).T).astype(E4NP)
    return np.ascontiguousarray(
        wT.reshape(NH, 2, P, NCC, P).transpose(0, 2, 3, 1, 4)
    ).reshape(NH * P, NCC, 2, P)


def _pack8_mov(w):
    # moving: [h*P+p, t, o] = fp8(w.T[128*(2h+t)+p, o])
    wT = np.ascontiguousarray(np.asarray(w, np.float32).T).astype(E4NP)
    return np.ascontiguousarray(
        wT.reshape(NH, 2, P, C).transpose(0, 2, 1, 3)
    ).reshape(NH * P, 2, C)


def make_in_maps(inputs):
    x = np.asarray(inputs["x"], np.float32).reshape(B, C, HW)
    shared = {
        "wq8": _pack8_stat(inputs["wq"]),
        "wk8": _pack8_stat(inputs["wk"]),
        "wv8": _pack8_mov(inputs["wv"]),
        "wo8": _pack8_stat(inputs["wo"]),
        "gnw": np.ascontiguousarray(np.asarray(inputs["gn_w"], np.float32)),
        "gnb": np.ascontiguousarray(np.asarray(inputs["gn_b"], np.float32)),
        "membd": _memb_np(),
        "bcd": _bc_np(),
    }
    in_maps = []
    for core in range(8):
        b, h = core // 2, core % 2
        xbm = x[b]
        if h == 1:
            xbm = np.concatenate([xbm[:, HALF:], xbm[:, :HALF]], axis=1)
        in_maps.append({"xb": np.ascontiguousarray(xbm), **shared})
    return in_maps


def assemble(results):
    out = np.empty((B, C, HW), np.float32)
    for core in range(8):
        b, h = core // 2, core % 2
        out[b][:, h * HALF : (h + 1) * HALF] = results[core]["out"]
    return out.reshape(B, C, H, W)


def run_spmd(inputs, trace=False):
    nc = _get_module()
    res = run_bass_kernel_spmd(
        nc, make_in_maps(inputs), core_ids=list(range(8)), trace=trace
    )
    return assemble(res.results), res


def kernel(**inputs) -> np.ndarray:
    out, _ = run_spmd(inputs)
    return out


# revision 54
# speedup vs baseline: 1.0264x; 1.0264x over previous
"""AttnBlock (GroupNorm -> q/k/v 1x1 conv -> full spatial attention -> out proj
-> residual) for Trainium2, sharded over 8 NeuronCores.

Sharding: 8 cores = 4 batches x 2 query-halves. Each core gets its batch's
full x (columns rotated so its 2048 query positions come first), computes
GroupNorm + k/v over all 4096 positions (k/v redundantly per half) and
attention output for its 2048 queries.

On-chip design (per core), all big matmuls in fp8-e4m3 with DoubleRow perf
mode (contracts 256 rows/instruction = 2x the f32r rate; measured 223ns per
[256K x 128 x 512] instruction vs 233ns for f32r's 128K):
  - x is DMA'd once into SBUF and stays resident (stats pass reads SBUF-bound
    tiles as they land; projections and the residual reuse it -> no second
    HBM pass, no q DRAM scratch).
  - weights are quantized to fp8 on the host directly in the DoubleRow pair
    layout [h][c128, 2, o] and DMA'd straight to SBUF.
  - xn is quantized to fp8 by DVE tensor_scalar (gscale*x+gshift) into pair
    layout; q/k (feature-major) and vT (token-major) projections are
    DoubleRow matmuls; PSUM tiles drain to fp8 SBUF via ACT/DVE copies.
  - attention per 512-query group: logits (2 insts/jc) -> ACT exp with
    bias -3 (softmax-invariant shift keeping exp in e4m3 range) written as
    fp8 -> PV DoubleRow over jc pairs into 4 held PSUM banks. The softmax
    denominator accumulates on the PE via a 32-column ones DoubleRow
    matmul per pair (frees DVE from the inner loop; M=1 stationaries are
    illegal in dual-fp8). Logit fronts are emitted two pairs ahead of
    their dps/pv consumers so the PE never waits on ACT (99-100% PE busy
    in steady state). pv drains are scaled by 2^-6 into fp8; normalization
    (64/denom, folded into the ones=64 broadcast matmul) is deferred past
    the fp8 out-projection, then the residual is added from resident x.
    Each group's tail is spliced into the next group's matmul stream.
  - outputs DMA on gpsimd's hardware queues while attention runs (sync
    software-queue bursts steal SBUF ports from the PE and slow the matmul
    stream ~10%); the last two groups flush on sync, which drains faster.

Measured on HW: 243.7us vs the 457.7us f32r baseline (1.89x), rel err
8.2e-3 against the f32 reference (gate 2e-2).
"""

import numpy as np
import ml_dtypes

import bass_rust
import concourse.bass as bass
import concourse.tile as tile
from concourse import mybir
from concourse.bass_utils import run_bass_kernel_spmd

B, C, H, W = 4, 512, 64, 64
HW = H * W            # 4096
HALF = HW // 2        # 2048 query positions per core
NG = 32               # groups
GS = C // NG          # 16 channels per group
EPS = 1e-6
P = 128               # SBUF partitions
NCC = C // P          # 4 channel chunks
NH = NCC // 2         # 2 channel-chunk pairs (DoubleRow)
JT = 512              # projection j-tile width
NJT = HW // JT        # 8
NJC = HW // P         # 32 j-chunks of 128
NPAIR = NJC // 2      # 16 j-chunk pairs
IGW = 512             # query-group width
NIG = HALF // IGW     # 4
SCALE = 1.0 / float(np.sqrt(C))
EXP_BIAS = -3.0       # softmax-invariant logit shift: exp stays in e4m3 range
PV_SCALE = 1.0 / 64.0  # raw-pv prescale into fp8; 64 folded into ones-bcast
F32 = mybir.dt.float32
F32R = mybir.dt.float32r
F8 = mybir.dt.float8e4
DR = mybir.MatmulPerfMode.DoubleRow
E4NP = ml_dtypes.float8_e4m3

AF = mybir.ActivationFunctionType
ALU = mybir.AluOpType


def _split_drain_waits(nc, max_waits=1):
    """walrus on this container rejects ANY instruction carrying more than one
    sem wait; spill the excess onto same-engine NoOps inserted just before
    (the engine executes the NoOp's waits, then the instruction's remaining
    one -- identical semantics)."""
    uid = [0]
    nsplit = 0
    for f in nc.m.functions:
        for bb in f.blocks:
            insts = bb.instructions
            i = 0
            while i < len(insts):
                inst = insts[i]
                si = getattr(inst, "sync_info", None)
                if si is not None and si.on_wait and len(si.on_wait) > max_waits:
                    waits = list(si.on_wait)
                    keep, rest = waits[-max_waits:], waits[:-max_waits]
                    new_insts = []
                    for j in range(0, len(rest), max_waits):
                        nop = mybir.InstNoOp(
                            name=f"wait-split-{uid[0]}", ins=[], outs=[]
                        )
                        uid[0] += 1
                        nop.engine = inst.engine
                        nop.sync_info = bass_rust.SyncInfo(
                            on_wait=rest[j : j + max_waits], on_update=[]
                        )
                        new_insts.append(nop)
                    inst.sync_info = bass_rust.SyncInfo(
                        on_wait=keep, on_update=list(si.on_update)
                    )
                    for k, nop in enumerate(new_insts):
                        insts.insert(i + k, nop)
                    i += len(new_insts)
                    nsplit += 1
                i += 1
    return nsplit


def build():
    nc = bass.Bass()
    xb = nc.dram_tensor("xb", [C, HW], F32, kind="ExternalInput")
    # stationary (lhsT) weights: [p, oc, t, m] -- each (t, m) pair block is
    # contiguous per partition (dual-fp8 LDWEIGHTS ISA requirement)
    wq8 = nc.dram_tensor("wq8", [NH * P, NCC, 2, P], F8, kind="ExternalInput")
    wk8 = nc.dram_tensor("wk8", [NH * P, NCC, 2, P], F8, kind="ExternalInput")
    wo8 = nc.dram_tensor("wo8", [NH * P, NCC, 2, P], F8, kind="ExternalInput")
    # moving (rhs) weights for the v projection: [p, t, o]
    wv8 = nc.dram_tensor("wv8", [NH * P, 2, C], F8, kind="ExternalInput")
    gnw = nc.dram_tensor("gnw", [C], F32, kind="ExternalInput")
    gnb = nc.dram_tensor("gnb", [C], F32, kind="ExternalInput")
    # local (within-chunk) group membership: identical for every channel
    # chunk since groups never span chunks (8 groups x 16 channels per chunk)
    membd = nc.dram_tensor("membd", [P, 8], F32, kind="ExternalInput")
    bcd = nc.dram_tensor("bcd", [8, P], F32, kind="ExternalInput")
    outd = nc.dram_tensor("out", [C, HALF], F32, kind="ExternalOutput")

    with tile.TileContext(nc) as tc, nc.allow_low_precision(
        reason="fp8 matmul pipeline; rel gate 2e-2, measured ~8e-3"
    ):
        with tc.tile_pool(name="pers", bufs=1) as pers:
            # ---- persistent tiles ----
            x_sb = [pers.tile([P, HW], F32, tag=f"x{cc}", name=f"x{cc}")
                    for cc in range(NCC)]
            # stationary tiles: [p, blk, t, m] -- (t, m) contiguous per blk
            k8_sb = [pers.tile([P, NJC, 2, P], F8, tag=f"k8{h}", name=f"k8{h}")
                     for h in range(NH)]
            vT8_sb = pers.tile([P, NPAIR, NCC, 2, P], F8, tag="vT8")
            q8_sb = [pers.tile([P, 2, HALF], F8, tag=f"q8{h}", name=f"q8{h}")
                     for h in range(NH)]
            wq_sb = [pers.tile([P, NCC, 2, P], F8, tag=f"wq{h}", name=f"wq{h}")
                     for h in range(NH)]
            wk_sb = [pers.tile([P, NCC, 2, P], F8, tag=f"wk{h}", name=f"wk{h}")
                     for h in range(NH)]
            wv_sb = [pers.tile([P, 2, C], F8, tag=f"wv{h}", name=f"wv{h}")
                     for h in range(NH)]
            wo_sb = [pers.tile([P, NCC, 2, P], F8, tag=f"wo{h}", name=f"wo{h}")
                     for h in range(NH)]
            attn8 = [pers.tile([P, 2, IGW], F8, tag=f"at8{h}", name=f"at8{h}")
                     for h in range(NH)]
            xn8_first = [pers.tile([P, 8, 2, P], F8, tag=f"xnf{h}", name=f"xnf{h}")
                         for h in range(NH)]
            # weight/const DMAs: keep them OFF the scalar queue (ACT does the
            # stats Squares on the critical path); gn/memb/bc go early on
            # sync, the fp8 weights on gpsimd interleaved with x tiles.
            gnw_t = pers.tile([P, NCC], F32, tag="gnw")
            gnb_t = pers.tile([P, NCC], F32, tag="gnb")
            nc.gpsimd.dma_start(out=gnw_t, in_=gnw.ap().rearrange("(a p) -> p a", p=P))
            nc.gpsimd.dma_start(out=gnb_t, in_=gnb.ap().rearrange("(a p) -> p a", p=P))
            gscale = pers.tile([P, NCC], F32, tag="gsc")
            gshift = pers.tile([P, NCC], F32, tag="gsh")
            memb = pers.tile([P, 8], F32, tag="memb")
            nc.gpsimd.dma_start(out=memb, in_=membd.ap())
            bc = pers.tile([8, P], F32, tag="bc")
            nc.gpsimd.dma_start(out=bc, in_=bcd.ap())
            ones1r64f = pers.tile([1, P], F32, tag="ones64f")
            nc.vector.memset(ones1r64f, 64.0)
            ones1r64 = pers.tile([1, P], F32R, tag="ones64")
            nc.vector.tensor_copy(out=ones1r64, in_=ones1r64f)
            ones8f = pers.tile([P, 2, 32], F32, tag="ones8f")
            nc.vector.memset(ones8f, 1.0)
            ones8 = pers.tile([P, 2, 32], F8, tag="ones8")
            nc.vector.tensor_copy(out=ones8, in_=ones8f)
            ebias = pers.tile([P, 1], F32, tag="ebias")
            nc.vector.memset(ebias, EXP_BIAS)

            # ---- phase A: x -> SBUF once; groupnorm statistics on the fly ----
            with (
                tc.tile_pool(name="statq", bufs=2) as sq_pool,
                tc.tile_pool(name="statsm", bufs=1) as sm,
                tc.tile_pool(name="statps", bufs=1, space="PSUM") as sps,
                nc.named_scope("stats"),
            ):
                sbeps = sm.tile([8, 1], F32, tag="eps")
                nc.vector.memset(sbeps, EPS)
                for cc in range(NCC):
                    s1t = sm.tile([P, 4], F32, tag=f"s1{cc}", name=f"s1{cc}")
                    s2t = sm.tile([P, 4], F32, tag=f"s2{cc}", name=f"s2{cc}")
                    for jt in range(4):
                        # all x tiles on the sync (software-dynamic) DMA
                        # queues: they complete far sooner than the gpsimd
                        # hardware queues, and the stats chain gates on the
                        # slowest stream
                        xsl = x_sb[cc][:, jt * 1024 : (jt + 1) * 1024]
                        nc.sync.dma_start(
                            out=xsl,
                            in_=xb.ap()[
                                cc * P : (cc + 1) * P,
                                jt * 1024 : (jt + 1) * 1024,
                            ],
                        )
                        nc.vector.reduce_sum(
                            out=s1t[:, jt : jt + 1], in_=xsl, axis=mybir.AxisListType.X
                        )
                        sqw = sq_pool.tile([P, 1024], F32, tag="sqw", name="sqw")
                        nc.scalar.activation(
                            out=sqw,
                            in_=xsl,
                            func=AF.Square,
                            accum_out=s2t[:, jt : jt + 1],
                        )
                    # groups never span channel chunks (8 groups per chunk),
                    # so each chunk's gscale/gshift resolves independently --
                    # no serial all-chunk reduction at the stats tail
                    mm2 = sm.tile([P, 2], F32, tag=f"m2{cc}", name=f"m2{cc}")
                    m1r = sm.tile([P, 1], F32, tag=f"m1r{cc}", name=f"m1r{cc}")
                    nc.vector.reduce_sum(out=m1r, in_=s1t, axis=mybir.AxisListType.X)
                    nc.vector.tensor_scalar_mul(mm2[:, 0:1], m1r, 1.0 / HW)
                    m2r = sm.tile([P, 1], F32, tag=f"m2r{cc}", name=f"m2r{cc}")
                    nc.vector.reduce_sum(out=m2r, in_=s2t, axis=mybir.AxisListType.X)
                    nc.vector.tensor_scalar_mul(mm2[:, 1:2], m2r, 1.0 / HW)
                    gps = sps.tile([8, 2], F32, tag="gstat", name=f"gps{cc}")
                    nc.tensor.matmul(gps, memb, mm2, start=True, stop=True)
                    # group stats for this chunk's 8 local groups
                    gs = sm.tile([8, 2], F32, tag=f"gs{cc}", name=f"gs{cc}")
                    nc.scalar.mul(gs, gps, 1.0 / GS)
                    sqg = sm.tile([8, 1], F32, tag=f"sq{cc}", name=f"sq{cc}")
                    nc.vector.tensor_mul(sqg, gs[:, 0:1], gs[:, 0:1])
                    varg = sm.tile([8, 1], F32, tag=f"vr{cc}", name=f"vr{cc}")
                    nc.vector.tensor_sub(varg, gs[:, 1:2], sqg)
                    g2 = sm.tile([8, 2], F32, tag=f"g2{cc}", name=f"g2{cc}")
                    nc.vector.tensor_copy(g2[:, 0:1], gs[:, 0:1])
                    nc.scalar.activation(
                        out=g2[:, 1:2], in_=varg, func=AF.Sqrt, bias=sbeps
                    )
                    nc.vector.reciprocal(out=g2[:, 1:2], in_=g2[:, 1:2])
                    chp = sps.tile([P, 2], F32, tag="chs", name="chs")
                    nc.tensor.matmul(chp, bc, g2, start=True, stop=True)
                    nc.vector.tensor_mul(
                        gscale[:, cc : cc + 1], chp[:, 1:2], gnw_t[:, cc : cc + 1]
                    )
                    tmpm = sm.tile([P, 1], F32, tag="tm", name="tm")
                    nc.vector.tensor_mul(tmpm, chp[:, 0:1], gscale[:, cc : cc + 1])
                    nc.vector.tensor_sub(
                        gshift[:, cc : cc + 1], gnb_t[:, cc : cc + 1], tmpm
                    )
                    # quantize this chunk's first projection supertile now --
                    # DVE would otherwise serialize all four quants right
                    # before the first projection matmul
                    nc.vector.tensor_scalar(
                        out=xn8_first[cc // 2][:, :, cc % 2, :],
                        in0=x_sb[cc][:, 0 : 2 * JT].rearrange(
                            "p (a m) -> p a m", a=8
                        ),
                        scalar1=gscale[:, cc : cc + 1],
                        scalar2=gshift[:, cc : cc + 1],
                        op0=ALU.mult,
                        op1=ALU.add,
                    )
                # fp8 weights land behind the x stream on gpsimd (needed
                # only once projections start)
                for h in range(NH):
                    wsl = slice(h * P, (h + 1) * P)
                    nc.gpsimd.dma_start(out=wq_sb[h], in_=wq8.ap()[wsl])
                    nc.gpsimd.dma_start(out=wk_sb[h], in_=wk8.ap()[wsl])
                    nc.gpsimd.dma_start(out=wv_sb[h], in_=wv8.ap()[wsl])
                    nc.gpsimd.dma_start(out=wo_sb[h], in_=wo8.ap()[wsl])

            # ---- phase B: projections (k, vT, q), all fp8 DoubleRow ----
            with (
                tc.tile_pool(name="projxn", bufs=2) as pxn,
                tc.tile_pool(name="projps", bufs=4, space="PSUM") as pps,
                nc.named_scope("proj"),
            ):
                # drains batch an output-channel (or token-chunk) pair into
                # one [128, 1024] copy spanning two PSUM banks -- amortizes
                # the per-op access latency on ACT/DVE
                for jt2 in range(NJT // 2):
                    jsl2 = slice(jt2 * 2 * JT, (jt2 + 1) * 2 * JT)
                    # xn8[h]: [p, js(8), t, m] -- contiguous (t, m) pair
                    # blocks for the v lhsT; q/k use the permuted view.
                    # jt2==0 was already quantized during the stats tail.
                    if jt2 == 0:
                        xn8 = xn8_first
                    else:
                        xn8 = [pxn.tile([P, 8, 2, P], F8, tag=f"xn{h}", name=f"xn{h}")
                               for h in range(NH)]
                        for cc in range(NCC):
                            nc.vector.tensor_scalar(
                                out=xn8[cc // 2][:, :, cc % 2, :],
                                in0=x_sb[cc][:, jsl2].rearrange("p (a m) -> p a m", a=8),
                                scalar1=gscale[:, cc : cc + 1],
                                scalar2=gshift[:, cc : cc + 1],
                                op0=ALU.mult,
                                op1=ALU.add,
                            )
                    for half in range(2):
                        jt = jt2 * 2 + half
                        jsl = slice(jt * JT, (jt + 1) * JT)
                        xnmov = [
                            xn8[h].rearrange("p a t m -> p t a m")[
                                :, :, half * 4 : (half + 1) * 4, :
                            ]
                            for h in range(NH)
                        ]
                        # k pairs (feature-major)
                        for hp in range(NH):
                            kps = pps.tile([P, 2 * JT], F32, tag="pp", name="kps")
                            for t in range(2):
                                oc = 2 * hp + t
                                for h in range(NH):
                                    nc.tensor.matmul(
                                        kps[:, t * JT : (t + 1) * JT],
                                        wk_sb[h][:, oc, :, :],
                                        xnmov[h],
                                        start=(h == 0),
                                        stop=(h == NH - 1),
                                        perf_mode=DR,
                                    )
                            nc.scalar.copy(
                                out=k8_sb[hp][:, jt * 4 : (jt + 1) * 4, :, :],
                                in_=kps.rearrange("p (t a m) -> p a t m", t=2, a=4),
                            )
                        # v pairs (token-major)
                        for vp in range(2):
                            vg = jt * 2 + vp
                            vps = pps.tile([P, 2 * JT], F32, tag="pp", name="vps")
                            for t in range(2):
                                js8 = half * 4 + 2 * vp + t
                                for h in range(NH):
                                    nc.tensor.matmul(
                                        vps[:, t * JT : (t + 1) * JT],
                                        xn8[h][:, js8, :, :],
                                        wv_sb[h],
                                        start=(h == 0),
                                        stop=(h == NH - 1),
                                        perf_mode=DR,
                                    )
                            vdst = vT8_sb[:, vg, :, :, :]
                            vsrc = vps.rearrange("p (t a m) -> p a t m", t=2, a=4)
                            # jt2 0-1 carry the q drains on ACT, so v goes to
                            # DVE there; later supertiles split v evenly
                            if jt2 < 2 or vp == 1:
                                nc.vector.tensor_copy(out=vdst, in_=vsrc)
                            else:
                                nc.scalar.copy(out=vdst, in_=vsrc)
                        # q pairs (first half only = our queries) on DVE
                        if jt < NJT // 2:
                            for hp in range(NH):
                                qps = pps.tile([P, 2 * JT], F32, tag="pp", name="qps")
                                for t in range(2):
                                    oc = 2 * hp + t
                                    for h in range(NH):
                                        nc.tensor.matmul(
                                            qps[:, t * JT : (t + 1) * JT],
                                            wq_sb[h][:, oc, :, :],
                                            xnmov[h],
                                            start=(h == 0),
                                            stop=(h == NH - 1),
                                            perf_mode=DR,
                                        )
                                nc.scalar.copy(
                                    out=q8_sb[hp][:, :, jsl],
                                    in_=qps.rearrange("p (t m) -> p t m", t=2),
                                )

            # ---- phase C: attention + output projection + residual ----
            with (
                tc.tile_pool(name="attnex", bufs=4) as aep,
                tc.tile_pool(name="attnsm", bufs=2) as asm_,
                tc.tile_pool(name="attnfo", bufs=3) as afo,
                tc.tile_pool(name="attnap", bufs=2, space="PSUM") as ap2,
                tc.tile_pool(name="attnpv", bufs=1, space="PSUM") as pvp_pool,
                tc.tile_pool(name="attndn", bufs=2, space="PSUM") as dnp,
                nc.named_scope("attn"),
            ):
                pending = None
                for ig in range(NIG):
                    isl = slice(ig * IGW, (ig + 1) * IGW)
                    dps = dnp.tile([32, IGW], F32, tag="dps", name="dps")
                    pvp = [
                        pvp_pool.tile([P, IGW], F32, tag=f"pv{cc}", name=f"pv{cc}")
                        for cc in range(NCC)
                    ]
                    exs = {}

                    # logits + exp for one key chunk; emitted two pairs ahead
                    # of its dps/pv consumers so the PE never waits on ACT
                    def emit_front(jc, isl=isl, exs=exs):
                        if pending is not None and jc in pending:
                            pending.pop(jc)()
                        ap_t = ap2.tile([P, IGW], F32, tag="ap", name="ap_t")
                        for h in range(NH):
                            nc.tensor.matmul(
                                ap_t,
                                k8_sb[h][:, jc, :, :],
                                q8_sb[h][:, :, isl],
                                start=(h == 0),
                                stop=(h == NH - 1),
                                perf_mode=DR,
                            )
                        if jc % 2 == 0:
                            exs[jc // 2] = aep.tile(
                                [P, 2, IGW], F8, tag="ex", name="ex"
                            )
                        nc.scalar.activation(
                            out=exs[jc // 2][:, jc % 2, :],
                            in_=ap_t,
                            func=AF.Exp,
                            scale=SCALE,
                            bias=ebias,
                        )

                    for jc in range(4):
                        emit_front(jc)
                    for pair in range(NPAIR):
                        if pair + 2 < NPAIR:
                            emit_front(2 * pair + 4)
                            emit_front(2 * pair + 5)
                        ex_pair = exs.pop(pair)
                        nc.tensor.matmul(
                            dps,
                            ones8,
                            ex_pair,
                            start=(pair == 0),
                            stop=(pair == NPAIR - 1),
                            perf_mode=DR,
                        )
                        for cc in range(NCC):
                            nc.tensor.matmul(
                                pvp[cc],
                                vT8_sb[:, pair, cc, :, :],
                                ex_pair,
                                start=(pair == 0),
                                stop=(pair == NPAIR - 1),
                                perf_mode=DR,
                            )
                    # ig end: drain raw pv (scaled 2^-6) into fp8 on DVE;
                    # frees the 4 pv banks for the next ig's first pair.
                    # (ACT drains here stall the next ig's exp chain -- only
                    # the final ig, with no exps after it, may use ACT, which
                    # overlaps the drains with the tail's reciprocal on DVE.)
                    for cc in range(NCC):
                        adst = attn8[cc // 2][:, cc % 2, :]
                        if ig == NIG - 1 and cc % 2 == 1:
                            nc.scalar.activation(
                                out=adst, in_=pvp[cc], func=AF.Copy,
                                scale=PV_SCALE,
                            )
                        else:
                            nc.vector.tensor_scalar_mul(adst, pvp[cc], PV_SCALE)

                    def make_tail(isl=isl, dps=dps, last=(ig >= NIG - 2)):
                        recip = asm_.tile([1, IGW], F32R, tag="recip", name="recip")
                        bcs = asm_.tile([P, IGW], F32, tag="bcs", name="bcs")

                        def t_norm():
                            # 64/denom broadcast to all partitions
                            nc.vector.reciprocal(out=recip, in_=dps[0:1, :])
                            bcp = ap2.tile([P, IGW], F32, tag="ap", name="bcp")
                            nc.tensor.matmul(
                                bcp, ones1r64, recip, start=True, stop=True
                            )
                            nc.vector.tensor_copy(out=bcs, in_=bcp)

                        def t_oc(oc):
                            def f():
                                oop = ap2.tile([P, IGW], F32, tag="ap", name="oop")
                                for h in range(NH):
                                    nc.tensor.matmul(
                                        oop,
                                        wo_sb[h][:, oc, :, :],
                                        attn8[h],
                                        start=(h == 0),
                                        stop=(h == NH - 1),
                                        perf_mode=DR,
                                    )
                                tmpo = afo.tile([P, IGW], F32, tag="tmpo", name="tmpo")
                                nc.vector.tensor_mul(tmpo, oop, bcs)
                                fo = afo.tile([P, IGW], F32, tag="fout", name="fout")
                                nc.vector.tensor_add(fo, tmpo, x_sb[oc][:, isl])
                                # gpsimd hw-queues while attention still runs
                                # (sync DMA bursts steal SBUF ports from the
                                # PE); the final group flushes on sync's fast
                                # software queues instead
                                oeng = nc.sync if last else nc.gpsimd
                                oeng.dma_start(
                                    out=outd.ap()[oc * P : (oc + 1) * P, isl],
                                    in_=fo,
                                )
                            return f

                        return {
                            5: t_norm,
                            8: t_oc(0),
                            10: t_oc(1),
                            12: t_oc(2),
                            14: t_oc(3),
                        }

                    pending = make_tail()
                for jc in sorted(pending):
                    pending[jc]()

    return nc


_NC_CACHE = {}


def _get_module():
    if "nc" not in _NC_CACHE:
        nc = build()
        _split_drain_waits(nc)  # only needed for walrus codegen, not CoreSim
        _NC_CACHE["nc"] = nc
    return _NC_CACHE["nc"]


def _memb_np():
    m = np.zeros((P, 8), np.float32)
    for p in range(P):
        m[p, p // GS] = 1.0
    return m


def _bc_np():
    b = np.zeros((8, P), np.float32)
    for p in range(P):
        b[p // GS, p] = 1.0
    return b


def _pack8_stat(w):
    # stationary: [h*P+p, oc, t, m] = fp8(w.T[128*(2h+t)+p, 128*oc+m])
    wT = np.ascontiguousarray(np.asarray(w, np.float32).T).astype(E4NP)
    return np.ascontiguousarray(
        wT.reshape(NH, 2, P, NCC, P).transpose(0, 2, 3, 1, 4)
    ).reshape(NH * P, NCC, 2, P)


def _pack8_mov(w):
    # moving: [h*P+p, t, o] = fp8(w.T[128*(2h+t)+p, o])
    wT = np.ascontiguousarray(np.asarray(w, np.float32).T).astype(E4NP)
    return np.ascontiguousarray(
        wT.reshape(NH, 2, P, C).transpose(0, 2, 1, 3)
    ).reshape(NH * P, 2, C)


def make_in_maps(inputs):
    x = np.asarray(inputs["x"], np.float32).reshape(B, C, HW)
    shared = {
        "wq8": _pack8_stat(inputs["wq"]),
        "wk8": _pack8_stat(inputs["wk"]),
        "wv8": _pack8_mov(inputs["wv"]),
        "wo8": _pack8_stat(inputs["wo"]),
        "gnw": np.ascontiguousarray(np.asarray(inputs["gn_w"], np.float32)),
        "gnb": np.ascontiguousarray(np.asarray(inputs["gn_b"], np.float32)),
        "membd": _memb_np(),
        "bcd": _bc_np(),
    }
    in_maps = []
    for core in range(8):
        b, h = core // 2, core % 2
        xbm = x[b]
        if h == 1:
            xbm = np.concatenate([xbm[:, HALF:], xbm[:, :HALF]], axis=1)
        in_maps.append({"xb": np.ascontiguousarray(xbm), **shared})
    return in_maps


def assemble(results):
    out = np.empty((B, C, HW), np.float32)
    for core in range(8):
        b, h = core // 2, core % 2
        out[b][:, h * HALF : (h + 1) * HALF] = results[core]["out"]
    return out.reshape(B, C, H, W)


def run_spmd(inputs, trace=False):
    nc = _get_module()
    res = run_bass_kernel_spmd(
        nc, make_in_maps(inputs), core_ids=list(range(8)), trace=trace
    )
    return assemble(res.results), res


def kernel(**inputs) -> np.ndarray:
    out, _ = run_spmd(inputs)
    return out


# revision 55
# speedup vs baseline: 1.0370x; 1.0103x over previous
"""AttnBlock (GroupNorm -> q/k/v 1x1 conv -> full spatial attention -> out proj
-> residual) for Trainium2, sharded over 8 NeuronCores.

Sharding: 8 cores = 4 batches x 2 query-halves. Each core gets its batch's
full x (columns rotated so its 2048 query positions come first), computes
GroupNorm + k/v over all 4096 positions (k/v redundantly per half) and
attention output for its 2048 queries.

On-chip design (per core), all big matmuls in fp8-e4m3 with DoubleRow perf
mode (contracts 256 rows/instruction = 2x the f32r rate; measured 223ns per
[256K x 128 x 512] instruction vs 233ns for f32r's 128K):
  - x is DMA'd once into SBUF and stays resident (stats pass reads SBUF-bound
    tiles as they land; projections and the residual reuse it -> no second
    HBM pass, no q DRAM scratch).
  - weights are quantized to fp8 on the host directly in the DoubleRow pair
    layout [h][c128, 2, o] and DMA'd straight to SBUF.
  - xn is quantized to fp8 by DVE tensor_scalar (gscale*x+gshift) into pair
    layout; q/k (feature-major) and vT (token-major) projections are
    DoubleRow matmuls; PSUM tiles drain to fp8 SBUF via ACT/DVE copies.
  - attention per 512-query group: logits (2 insts/jc) -> ACT exp with
    bias -3 (softmax-invariant shift keeping exp in e4m3 range) written as
    fp8 -> PV DoubleRow over jc pairs into 4 held PSUM banks. The softmax
    denominator accumulates on the PE via a 32-column ones DoubleRow
    matmul per pair (frees DVE from the inner loop; M=1 stationaries are
    illegal in dual-fp8). Logit fronts are emitted two pairs ahead of
    their dps/pv consumers so the PE never waits on ACT (99-100% PE busy
    in steady state). pv drains are scaled by 2^-6 into fp8; normalization
    (64/denom, folded into the ones=64 broadcast matmul) is deferred past
    the fp8 out-projection, then the residual is added from resident x.
    Each group's tail is spliced into the next group's matmul stream.
  - outputs DMA on gpsimd's hardware queues while attention runs (sync
    software-queue bursts steal SBUF ports from the PE and slow the matmul
    stream ~10%); the last two groups flush on sync, which drains faster.

Measured on HW: 243.7us vs the 457.7us f32r baseline (1.89x), rel err
8.2e-3 against the f32 reference (gate 2e-2).
"""

import numpy as np
import ml_dtypes

import bass_rust
import concourse.bass as bass
import concourse.tile as tile
from concourse import mybir
from concourse.bass_utils import run_bass_kernel_spmd

B, C, H, W = 4, 512, 64, 64
HW = H * W            # 4096
HALF = HW // 2        # 2048 query positions per core
NG = 32               # groups
GS = C // NG          # 16 channels per group
EPS = 1e-6
P = 128               # SBUF partitions
NCC = C // P          # 4 channel chunks
NH = NCC // 2         # 2 channel-chunk pairs (DoubleRow)
JT = 512              # projection j-tile width
NJT = HW // JT        # 8
NJC = HW // P         # 32 j-chunks of 128
NPAIR = NJC // 2      # 16 j-chunk pairs
IGW = 512             # query-group width
NIG = HALF // IGW     # 4
SCALE = 1.0 / float(np.sqrt(C))
EXP_BIAS = -3.0       # softmax-invariant logit shift: exp stays in e4m3 range
PV_SCALE = 1.0 / 64.0  # raw-pv prescale into fp8; 64 folded into ones-bcast
F32 = mybir.dt.float32
F32R = mybir.dt.float32r
F8 = mybir.dt.float8e4
DR = mybir.MatmulPerfMode.DoubleRow
E4NP = ml_dtypes.float8_e4m3

AF = mybir.ActivationFunctionType
ALU = mybir.AluOpType


def _split_drain_waits(nc, max_waits=1):
    """walrus on this container rejects ANY instruction carrying more than one
    sem wait; spill the excess onto same-engine NoOps inserted just before
    (the engine executes the NoOp's waits, then the instruction's remaining
    one -- identical semantics)."""
    uid = [0]
    nsplit = 0
    for f in nc.m.functions:
        for bb in f.blocks:
            insts = bb.instructions
            i = 0
            while i < len(insts):
                inst = insts[i]
                si = getattr(inst, "sync_info", None)
                if si is not None and si.on_wait and len(si.on_wait) > max_waits:
                    waits = list(si.on_wait)
                    keep, rest = waits[-max_waits:], waits[:-max_waits]
                    new_insts = []
                    for j in range(0, len(rest), max_waits):
                        nop = mybir.InstNoOp(
                            name=f"wait-split-{uid[0]}", ins=[], outs=[]
                        )
                        uid[0] += 1
                        nop.engine = inst.engine
                        nop.sync_info = bass_rust.SyncInfo(
                            on_wait=rest[j : j + max_waits], on_update=[]
                        )
                        new_insts.append(nop)
                    inst.sync_info = bass_rust.SyncInfo(
                        on_wait=keep, on_update=list(si.on_update)
                    )
                    for k, nop in enumerate(new_insts):
                        insts.insert(i + k, nop)
                    i += len(new_insts)
                    nsplit += 1
                i += 1
    return nsplit


def build():
    nc = bass.Bass()
    xb = nc.dram_tensor("xb", [C, HW], F32, kind="ExternalInput")
    # stationary (lhsT) weights: [p, oc, t, m] -- each (t, m) pair block is
    # contiguous per partition (dual-fp8 LDWEIGHTS ISA requirement)
    wq8 = nc.dram_tensor("wq8", [NH * P, NCC, 2, P], F8, kind="ExternalInput")
    wk8 = nc.dram_tensor("wk8", [NH * P, NCC, 2, P], F8, kind="ExternalInput")
    wo8 = nc.dram_tensor("wo8", [NH * P, NCC, 2, P], F8, kind="ExternalInput")
    # moving (rhs) weights for the v projection: [p, t, o]
    wv8 = nc.dram_tensor("wv8", [NH * P, 2, C], F8, kind="ExternalInput")
    gnw = nc.dram_tensor("gnw", [C], F32, kind="ExternalInput")
    gnb = nc.dram_tensor("gnb", [C], F32, kind="ExternalInput")
    # local (within-chunk) group membership: identical for every channel
    # chunk since groups never span chunks (8 groups x 16 channels per chunk)
    membd = nc.dram_tensor("membd", [P, 8], F32, kind="ExternalInput")
    bcd = nc.dram_tensor("bcd", [8, P], F32, kind="ExternalInput")
    outd = nc.dram_tensor("out", [C, HALF], F32, kind="ExternalOutput")

    with tile.TileContext(nc) as tc, nc.allow_low_precision(
        reason="fp8 matmul pipeline; rel gate 2e-2, measured ~8e-3"
    ):
        with tc.tile_pool(name="pers", bufs=1) as pers:
            # ---- persistent tiles ----
            x_sb = [pers.tile([P, HW], F32, tag=f"x{cc}", name=f"x{cc}")
                    for cc in range(NCC)]
            # stationary tiles: [p, blk, t, m] -- (t, m) contiguous per blk
            k8_sb = [pers.tile([P, NJC, 2, P], F8, tag=f"k8{h}", name=f"k8{h}")
                     for h in range(NH)]
            vT8_sb = pers.tile([P, NPAIR, NCC, 2, P], F8, tag="vT8")
            q8_sb = [pers.tile([P, 2, HALF], F8, tag=f"q8{h}", name=f"q8{h}")
                     for h in range(NH)]
            wq_sb = [pers.tile([P, NCC, 2, P], F8, tag=f"wq{h}", name=f"wq{h}")
                     for h in range(NH)]
            wk_sb = [pers.tile([P, NCC, 2, P], F8, tag=f"wk{h}", name=f"wk{h}")
                     for h in range(NH)]
            wv_sb = [pers.tile([P, 2, C], F8, tag=f"wv{h}", name=f"wv{h}")
                     for h in range(NH)]
            wo_sb = [pers.tile([P, NCC, 2, P], F8, tag=f"wo{h}", name=f"wo{h}")
                     for h in range(NH)]
            attn8 = [pers.tile([P, 2, IGW], F8, tag=f"at8{h}", name=f"at8{h}")
                     for h in range(NH)]
            xn8_first = [pers.tile([P, 8, 2, P], F8, tag=f"xnf{h}", name=f"xnf{h}")
                         for h in range(NH)]
            # weight/const DMAs: keep them OFF the scalar queue (ACT does the
            # stats Squares on the critical path); gn/memb/bc go early on
            # sync, the fp8 weights on gpsimd interleaved with x tiles.
            gnw_t = pers.tile([P, NCC], F32, tag="gnw")
            gnb_t = pers.tile([P, NCC], F32, tag="gnb")
            nc.gpsimd.dma_start(out=gnw_t, in_=gnw.ap().rearrange("(a p) -> p a", p=P))
            nc.gpsimd.dma_start(out=gnb_t, in_=gnb.ap().rearrange("(a p) -> p a", p=P))
            gscale = pers.tile([P, NCC], F32, tag="gsc")
            gshift = pers.tile([P, NCC], F32, tag="gsh")
            memb = pers.tile([P, 8], F32, tag="memb")
            nc.gpsimd.dma_start(out=memb, in_=membd.ap())
            bc = pers.tile([8, P], F32, tag="bc")
            nc.gpsimd.dma_start(out=bc, in_=bcd.ap())
            ones1r64f = pers.tile([1, P], F32, tag="ones64f")
            nc.vector.memset(ones1r64f, 64.0)
            ones1r64 = pers.tile([1, P], F32R, tag="ones64")
            nc.vector.tensor_copy(out=ones1r64, in_=ones1r64f)
            ones8f = pers.tile([P, 2, 32], F32, tag="ones8f")
            nc.vector.memset(ones8f, 1.0)
            ones8 = pers.tile([P, 2, 32], F8, tag="ones8")
            nc.vector.tensor_copy(out=ones8, in_=ones8f)
            ebias = pers.tile([P, 1], F32, tag="ebias")
            nc.vector.memset(ebias, EXP_BIAS)

            # ---- phase A: x -> SBUF once; groupnorm statistics on the fly ----
            with (
                tc.tile_pool(name="statq", bufs=2) as sq_pool,
                tc.tile_pool(name="statsm", bufs=1) as sm,
                tc.tile_pool(name="statps", bufs=1, space="PSUM") as sps,
                nc.named_scope("stats"),
            ):
                sbeps = sm.tile([8, 1], F32, tag="eps")
                nc.vector.memset(sbeps, EPS)
                for cc in range(NCC):
                    s1t = sm.tile([P, 4], F32, tag=f"s1{cc}", name=f"s1{cc}")
                    s2t = sm.tile([P, 4], F32, tag=f"s2{cc}", name=f"s2{cc}")
                    # all x tiles on the sync (software-dynamic) DMA queues:
                    # they complete far sooner than the gpsimd hardware
                    # queues, and the stats chain gates on the slowest
                    # stream. 2048-wide DMAs halve the ~0.9us-per-issue
                    # serialization on the sync queue engine; stats ops stay
                    # 1024-wide to chase each half-tile as it lands.
                    for dh in range(2):
                        nc.sync.dma_start(
                            out=x_sb[cc][:, dh * 2048 : (dh + 1) * 2048],
                            in_=xb.ap()[
                                cc * P : (cc + 1) * P,
                                dh * 2048 : (dh + 1) * 2048,
                            ],
                        )
                    for jt in range(4):
                        xsl = x_sb[cc][:, jt * 1024 : (jt + 1) * 1024]
                        nc.vector.reduce_sum(
                            out=s1t[:, jt : jt + 1], in_=xsl, axis=mybir.AxisListType.X
                        )
                        sqw = sq_pool.tile([P, 1024], F32, tag="sqw", name="sqw")
                        nc.scalar.activation(
                            out=sqw,
                            in_=xsl,
                            func=AF.Square,
                            accum_out=s2t[:, jt : jt + 1],
                        )
                    # groups never span channel chunks (8 groups per chunk),
                    # so each chunk's gscale/gshift resolves independently --
                    # no serial all-chunk reduction at the stats tail
                    mm2 = sm.tile([P, 2], F32, tag=f"m2{cc}", name=f"m2{cc}")
                    m1r = sm.tile([P, 1], F32, tag=f"m1r{cc}", name=f"m1r{cc}")
                    nc.vector.reduce_sum(out=m1r, in_=s1t, axis=mybir.AxisListType.X)
                    nc.vector.tensor_scalar_mul(mm2[:, 0:1], m1r, 1.0 / HW)
                    m2r = sm.tile([P, 1], F32, tag=f"m2r{cc}", name=f"m2r{cc}")
                    nc.vector.reduce_sum(out=m2r, in_=s2t, axis=mybir.AxisListType.X)
                    nc.vector.tensor_scalar_mul(mm2[:, 1:2], m2r, 1.0 / HW)
                    gps = sps.tile([8, 2], F32, tag="gstat", name=f"gps{cc}")
                    nc.tensor.matmul(gps, memb, mm2, start=True, stop=True)
                    # group stats for this chunk's 8 local groups
                    gs = sm.tile([8, 2], F32, tag=f"gs{cc}", name=f"gs{cc}")
                    nc.scalar.mul(gs, gps, 1.0 / GS)
                    sqg = sm.tile([8, 1], F32, tag=f"sq{cc}", name=f"sq{cc}")
                    nc.vector.tensor_mul(sqg, gs[:, 0:1], gs[:, 0:1])
                    varg = sm.tile([8, 1], F32, tag=f"vr{cc}", name=f"vr{cc}")
                    nc.vector.tensor_sub(varg, gs[:, 1:2], sqg)
                    g2 = sm.tile([8, 2], F32, tag=f"g2{cc}", name=f"g2{cc}")
                    nc.vector.tensor_copy(g2[:, 0:1], gs[:, 0:1])
                    nc.scalar.activation(
                        out=g2[:, 1:2], in_=varg, func=AF.Sqrt, bias=sbeps
                    )
                    nc.vector.reciprocal(out=g2[:, 1:2], in_=g2[:, 1:2])
                    chp = sps.tile([P, 2], F32, tag="chs", name="chs")
                    nc.tensor.matmul(chp, bc, g2, start=True, stop=True)
                    nc.vector.tensor_mul(
                        gscale[:, cc : cc + 1], chp[:, 1:2], gnw_t[:, cc : cc + 1]
                    )
                    tmpm = sm.tile([P, 1], F32, tag="tm", name="tm")
                    nc.vector.tensor_mul(tmpm, chp[:, 0:1], gscale[:, cc : cc + 1])
                    nc.vector.tensor_sub(
                        gshift[:, cc : cc + 1], gnb_t[:, cc : cc + 1], tmpm
                    )
                    # quantize this chunk's first projection supertile now --
                    # DVE would otherwise serialize all four quants right
                    # before the first projection matmul
                    nc.vector.tensor_scalar(
                        out=xn8_first[cc // 2][:, :, cc % 2, :],
                        in0=x_sb[cc][:, 0 : 2 * JT].rearrange(
                            "p (a m) -> p a m", a=8
                        ),
                        scalar1=gscale[:, cc : cc + 1],
                        scalar2=gshift[:, cc : cc + 1],
                        op0=ALU.mult,
                        op1=ALU.add,
                    )
                # fp8 weights land behind the x stream on gpsimd (needed
                # only once projections start)
                for h in range(NH):
                    wsl = slice(h * P, (h + 1) * P)
                    nc.gpsimd.dma_start(out=wq_sb[h], in_=wq8.ap()[wsl])
                    nc.gpsimd.dma_start(out=wk_sb[h], in_=wk8.ap()[wsl])
                    nc.gpsimd.dma_start(out=wv_sb[h], in_=wv8.ap()[wsl])
                    nc.gpsimd.dma_start(out=wo_sb[h], in_=wo8.ap()[wsl])

            # ---- phase B: projections (k, vT, q), all fp8 DoubleRow ----
            with (
                tc.tile_pool(name="projxn", bufs=2) as pxn,
                tc.tile_pool(name="projps", bufs=4, space="PSUM") as pps,
                nc.named_scope("proj"),
            ):
                # drains batch an output-channel (or token-chunk) pair into
                # one [128, 1024] copy spanning two PSUM banks -- amortizes
                # the per-op access latency on ACT/DVE
                for jt2 in range(NJT // 2):
                    jsl2 = slice(jt2 * 2 * JT, (jt2 + 1) * 2 * JT)
                    # xn8[h]: [p, js(8), t, m] -- contiguous (t, m) pair
                    # blocks for the v lhsT; q/k use the permuted view.
                    # jt2==0 was already quantized during the stats tail.
                    if jt2 == 0:
                        xn8 = xn8_first
                    else:
                        xn8 = [pxn.tile([P, 8, 2, P], F8, tag=f"xn{h}", name=f"xn{h}")
                               for h in range(NH)]
                        for cc in range(NCC):
                            nc.vector.tensor_scalar(
                                out=xn8[cc // 2][:, :, cc % 2, :],
                                in0=x_sb[cc][:, jsl2].rearrange("p (a m) -> p a m", a=8),
                                scalar1=gscale[:, cc : cc + 1],
                                scalar2=gshift[:, cc : cc + 1],
                                op0=ALU.mult,
                                op1=ALU.add,
                            )
                    for half in range(2):
                        jt = jt2 * 2 + half
                        jsl = slice(jt * JT, (jt + 1) * JT)
                        xnmov = [
                            xn8[h].rearrange("p a t m -> p t a m")[
                                :, :, half * 4 : (half + 1) * 4, :
                            ]
                            for h in range(NH)
                        ]
                        # k pairs (feature-major)
                        for hp in range(NH):
                            kps = pps.tile([P, 2 * JT], F32, tag="pp", name="kps")
                            for t in range(2):
                                oc = 2 * hp + t
                                for h in range(NH):
                                    nc.tensor.matmul(
                                        kps[:, t * JT : (t + 1) * JT],
                                        wk_sb[h][:, oc, :, :],
                                        xnmov[h],
                                        start=(h == 0),
                                        stop=(h == NH - 1),
                                        perf_mode=DR,
                                    )
                            nc.scalar.copy(
                                out=k8_sb[hp][:, jt * 4 : (jt + 1) * 4, :, :],
                                in_=kps.rearrange("p (t a m) -> p a t m", t=2, a=4),
                            )
                        # v pairs (token-major)
                        for vp in range(2):
                            vg = jt * 2 + vp
                            vps = pps.tile([P, 2 * JT], F32, tag="pp", name="vps")
                            for t in range(2):
                                js8 = half * 4 + 2 * vp + t
                                for h in range(NH):
                                    nc.tensor.matmul(
                                        vps[:, t * JT : (t + 1) * JT],
                                        xn8[h][:, js8, :, :],
                                        wv_sb[h],
                                        start=(h == 0),
                                        stop=(h == NH - 1),
                                        perf_mode=DR,
                                    )
                            vdst = vT8_sb[:, vg, :, :, :]
                            vsrc = vps.rearrange("p (t a m) -> p a t m", t=2, a=4)
                            # jt2 0-1 carry the q drains on ACT, so v goes to
                            # DVE there; later supertiles split v evenly
                            if jt2 < 2 or vp == 1:
                                nc.vector.tensor_copy(out=vdst, in_=vsrc)
                            else:
                                nc.scalar.copy(out=vdst, in_=vsrc)
                        # q pairs (first half only = our queries) on DVE
                        if jt < NJT // 2:
                            for hp in range(NH):
                                qps = pps.tile([P, 2 * JT], F32, tag="pp", name="qps")
                                for t in range(2):
                                    oc = 2 * hp + t
                                    for h in range(NH):
                                        nc.tensor.matmul(
                                            qps[:, t * JT : (t + 1) * JT],
                                            wq_sb[h][:, oc, :, :],
                                            xnmov[h],
                                            start=(h == 0),
                                            stop=(h == NH - 1),
                                            perf_mode=DR,
                                        )
                                nc.scalar.copy(
                                    out=q8_sb[hp][:, :, jsl],
                                    in_=qps.rearrange("p (t m) -> p t m", t=2),
                                )

            # ---- phase C: attention + output projection + residual ----
            with (
                tc.tile_pool(name="attnex", bufs=4) as aep,
                tc.tile_pool(name="attnsm", bufs=2) as asm_,
                tc.tile_pool(name="attnfo", bufs=3) as afo,
                tc.tile_pool(name="attnap", bufs=2, space="PSUM") as ap2,
                tc.tile_pool(name="attnpv", bufs=1, space="PSUM") as pvp_pool,
                tc.tile_pool(name="attndn", bufs=2, space="PSUM") as dnp,
                nc.named_scope("attn"),
            ):
                pending = None
                for ig in range(NIG):
                    isl = slice(ig * IGW, (ig + 1) * IGW)
                    dps = dnp.tile([32, IGW], F32, tag="dps", name="dps")
                    pvp = [
                        pvp_pool.tile([P, IGW], F32, tag=f"pv{cc}", name=f"pv{cc}")
                        for cc in range(NCC)
                    ]
                    exs = {}

                    # logits + exp for one key chunk; emitted two pairs ahead
                    # of its dps/pv consumers so the PE never waits on ACT
                    def emit_front(jc, isl=isl, exs=exs):
                        if pending is not None and jc in pending:
                            pending.pop(jc)()
                        ap_t = ap2.tile([P, IGW], F32, tag="ap", name="ap_t")
                        for h in range(NH):
                            nc.tensor.matmul(
                                ap_t,
                                k8_sb[h][:, jc, :, :],
                                q8_sb[h][:, :, isl],
                                start=(h == 0),
                                stop=(h == NH - 1),
                                perf_mode=DR,
                            )
                        if jc % 2 == 0:
                            exs[jc // 2] = aep.tile(
                                [P, 2, IGW], F8, tag="ex", name="ex"
                            )
                        nc.scalar.activation(
                            out=exs[jc // 2][:, jc % 2, :],
                            in_=ap_t,
                            func=AF.Exp,
                            scale=SCALE,
                            bias=ebias,
                        )

                    for jc in range(4):
                        emit_front(jc)
                    for pair in range(NPAIR):
                        if pair + 2 < NPAIR:
                            emit_front(2 * pair + 4)
                            emit_front(2 * pair + 5)
                        ex_pair = exs.pop(pair)
                        nc.tensor.matmul(
                            dps,
                            ones8,
                            ex_pair,
                            start=(pair == 0),
                            stop=(pair == NPAIR - 1),
                            perf_mode=DR,
                        )
                        for cc in range(NCC):
                            nc.tensor.matmul(
                                pvp[cc],
                                vT8_sb[:, pair, cc, :, :],
                                ex_pair,
                                start=(pair == 0),
                                stop=(pair == NPAIR - 1),
                                perf_mode=DR,
                            )
                    # ig end: drain raw pv (scaled 2^-6) into fp8 on DVE;
                    # frees the 4 pv banks for the next ig's first pair.
                    # (ACT drains here stall the next ig's exp chain -- only
                    # the final ig, with no exps after it, may use ACT, which
                    # overlaps the drains with the tail's reciprocal on DVE.)
                    for cc in range(NCC):
                        adst = attn8[cc // 2][:, cc % 2, :]
                        if ig == NIG - 1 and cc % 2 == 1:
                            nc.scalar.activation(
                                out=adst, in_=pvp[cc], func=AF.Copy,
                                scale=PV_SCALE,
                            )
                        else:
                            nc.vector.tensor_scalar_mul(adst, pvp[cc], PV_SCALE)

                    def make_tail(isl=isl, dps=dps, last=(ig >= NIG - 2)):
                        recip = asm_.tile([1, IGW], F32R, tag="recip", name="recip")
                        bcs = asm_.tile([P, IGW], F32, tag="bcs", name="bcs")

                        def t_norm():
                            # 64/denom broadcast to all partitions
                            nc.vector.reciprocal(out=recip, in_=dps[0:1, :])
                            bcp = ap2.tile([P, IGW], F32, tag="ap", name="bcp")
                            nc.tensor.matmul(
                                bcp, ones1r64, recip, start=True, stop=True
                            )
                            nc.vector.tensor_copy(out=bcs, in_=bcp)

                        def t_oc(oc):
                            def f():
                                oop = ap2.tile([P, IGW], F32, tag="ap", name="oop")
                                for h in range(NH):
                                    nc.tensor.matmul(
                                        oop,
                                        wo_sb[h][:, oc, :, :],
                                        attn8[h],
                                        start=(h == 0),
                                        stop=(h == NH - 1),
                                        perf_mode=DR,
                                    )
                                tmpo = afo.tile([P, IGW], F32, tag="tmpo", name="tmpo")
                                nc.vector.tensor_mul(tmpo, oop, bcs)
                                fo = afo.tile([P, IGW], F32, tag="fout", name="fout")
                                nc.vector.tensor_add(fo, tmpo, x_sb[oc][:, isl])
                                # gpsimd hw-queues while attention still runs
                                # (sync DMA bursts steal SBUF ports from the
                                # PE); the final group flushes on sync's fast
                                # software queues instead
                                oeng = nc.sync if last else nc.gpsimd
                                oeng.dma_start(
                                    out=outd.ap()[oc * P : (oc + 1) * P, isl],
                                    in_=fo,
                                )
                            return f

                        return {
                            5: t_norm,
                            8: t_oc(0),
                            10: t_oc(1),
                            12: t_oc(2),
                            14: t_oc(3),
                        }

                    pending = make_tail()
                for jc in sorted(pending):
                    pending[jc]()

    return nc


_NC_CACHE = {}


def _get_module():
    if "nc" not in _NC_CACHE:
        nc = build()
        _split_drain_waits(nc)  # only needed for walrus codegen, not CoreSim
        _NC_CACHE["nc"] = nc
    return _NC_CACHE["nc"]


def _memb_np():
    m = np.zeros((P, 8), np.float32)
    for p in range(P):
        m[p, p // GS] = 1.0
    return m


def _bc_np():
    b = np.zeros((8, P), np.float32)
    for p in range(P):
        b[p // GS, p] = 1.0
    return b


def _pack8_stat(w):
    # stationary: [h*P+p, oc, t, m] = fp8(w.T[128*(2h+t)+p, 128*oc+m])
    wT = np.ascontiguousarray(np.asarray(w, np.float32).T).astype(E4NP)
    return np.ascontiguousarray(
        wT.reshape(NH, 2, P, NCC, P).transpose(0, 2, 3, 1, 4)
    ).reshape(NH * P, NCC, 2, P)


def _pack8_mov(w):
    # moving: [h*P+p, t, o] = fp8(w.T[128*(2h+t)+p, o])
    wT = np.ascontiguousarray(np.asarray(w, np.float32).T).astype(E4NP)
    return np.ascontiguousarray(
        wT.reshape(NH, 2, P, C).transpose(0, 2, 1, 3)
    ).reshape(NH * P, 2, C)


def make_in_maps(inputs):
    x = np.asarray(inputs["x"], np.float32).reshape(B, C, HW)
    shared = {
        "wq8": _pack8_stat(inputs["wq"]),
        "wk8": _pack8_stat(inputs["wk"]),
        "wv8": _pack8_mov(inputs["wv"]),
        "wo8": _pack8_stat(inputs["wo"]),
        "gnw": np.ascontiguousarray(np.asarray(inputs["gn_w"], np.float32)),
        "gnb": np.ascontiguousarray(np.asarray(inputs["gn_b"], np.float32)),
        "membd": _memb_np(),
        "bcd": _bc_np(),
    }
    in_maps = []
    for core in range(8):
        b, h = core // 2, core % 2
        xbm = x[b]
        if h == 1:
            xbm = np.concatenate([xbm[:, HALF:], xbm[:, :HALF]], axis=1)
        in_maps.append({"xb": np.ascontiguousarray(xbm), **shared})
    return in_maps


def assemble(results):
    out = np.empty((B, C, HW), np.float32)
    for core in range(8):
        b, h = core // 2, core % 2
        out[b][:, h * HALF : (h + 1) * HALF] = results[core]["out"]
    return out.reshape(B, C, H, W)


def run_spmd(inputs, trace=False):
    nc = _get_module()
    res = run_bass_kernel_spmd(
        nc, make_in_maps(inputs), core_ids=list(range(8)), trace=trace
    )
    return assemble(res.results), res


def kernel(**inputs) -> np.ndarray:
    out, _ = run_spmd(inputs)
    return out
